# revision 30
# baseline (speedup 1.0000x reference)
"""RegionLoss (YOLOv2) Trainium2 kernel — 8-core batch-parallel SPMD.

Contract: kernel(**inputs) takes FULL inputs (output [32,425,76,76] f32,
target [32,250] f32) and returns the FULL scalar loss, matching
reference.region_loss. Internally: batch is sharded 4 images/core across 8
NeuronCores; each core computes its partial loss on device; host sums the 8
partials (the unshard step for a sum-reduced output).

Device algorithm per core (4 images):
 - Bulk stage: for every anchor cell (4*5*76*76 = 115,520 cells) compute
   sigmoid(conf)^2 * [max_t IoU(pred_cell, gt_t) <= 0.6] and sum it.
   Division-free exact test per (cell, gt), rescaled by 1/1.5 so the
   per-cell threshold is plain phw*phh:
     iou > 0.6  <=>  (relu(ox)*oy - 0.375*ga_t)/1.5 > phw*phh
   with ox = min(pxr,gxr)-max(pxl,gxl), oy likewise (exact min/max form).
   Layout: all 128 partitions, 32 per image (anchor boundaries crossed;
   the anchor log-size bias is pre-added to w/h on host), F=904 cells per
   partition, fp16. The gt loop is blocked 4-at-a-time: per gt the DVE
   runs 3 tensor_scalar min/max ops (x-side min fused with a -gxl shift
   so the x-max rides the Scalar engine as relu(pxl-gxl)), and per block
   the DVE runs 4 [128,4F] tensor_tensors (s1, s3, prod, acc-max) while
   the Scalar engine supplies relu(ox) and the threshold-subtract
   (Identity, scale=-1, per-partition bias). GpSimd stays idle during
   the loop - its SBUF port is shared with the DVE and concurrent
   streaming slows both ~5x (measured 455ns -> 2205ns per op).
 - Small stage: the <=50 matched cells per image (last-write-wins deduped
   on host) contribute coord/obj-conf/class-CE terms; the 85 channel
   values at those cells are gathered host-side (a layout/shard step -
   all math stays on device) and processed in f32 on <=256 partitions.

Everything derived from `target` (gt boxes, best anchors, scatter cells,
masks) is host-precomputed metadata passed as small input tensors, so one
input-independent NEFF serves all cores/batches.
"""

import math
import numpy as np

# ---- problem constants (hardcoded per contract) ----
NB, NH, NW = 32, 76, 76
NA, NCLS = 5, 80
MAXT = 50
ANCHORS = np.array([1.3221, 1.73145, 3.19275, 4.00944, 5.05587, 8.09892,
                    9.47112, 4.84053, 11.2364, 10.0071], dtype=np.float32)
AW = ANCHORS.reshape(NA, 2)[:, 0]
AH = ANCHORS.reshape(NA, 2)[:, 1]
COORD_SCALE, NOOBJ_SCALE, OBJ_SCALE, CLASS_SCALE = 1.0, 1.0, 5.0, 1.0
THRESH = 0.6

NCORES = 8
BPC = NB // NCORES          # 4 images per core
HW = NH * NW                # 5776
PPI = 128 // BPC            # 32 partitions per image
F = (NA * HW + PPI - 1) // PPI  # 904 cells per partition (28880 -> 28928)
NCELL_CAP = 256             # small-stage cell capacity (2 x 128)
BIG = 6.0e4                 # fp16-safe sentinel

_PROG_CACHE = {}


def _build_program():
    import concourse.bacc as bacc
    import concourse.mybir as mybir
    from concourse.tile import TileContext

    f32 = mybir.dt.float32
    f16 = mybir.dt.float16
    Alu = mybir.AluOpType
    Act = mybir.ActivationFunctionType
    X = mybir.AxisListType.X

    nc = bacc.Bacc()

    # ---- I/O ----
    chans = nc.declare_dram_parameter("chans", [5, 128, F], f16, isOutput=False)
    auxgt = nc.declare_dram_parameter("auxgt", [128, 7 * MAXT], f32, isOutput=False)
    grids = nc.declare_dram_parameter("grids", [128, 2 * F], f16, isOutput=False)
    maskc = nc.declare_dram_parameter("maskc", [128, F], f16, isOutput=False)
    gath = nc.declare_dram_parameter("gath", [NCELL_CAP, 85], f32, isOutput=False)
    auxc = nc.declare_dram_parameter("auxc", [NCELL_CAP, 16], f32, isOutput=False)
    oneh = nc.declare_dram_parameter("oneh", [NCELL_CAP, NCLS], f32, isOutput=False)
    out_d = nc.declare_dram_parameter("out", [1, 16], f32, isOutput=True)

    with TileContext(nc) as tc:
        with tc.tile_pool(name="per", bufs=1) as per, \
             tc.tile_pool(name="tmp", bufs=2) as tmp, \
             tc.tile_pool(name="ps", bufs=1, space="PSUM") as ps:

            # ---------- loads (critical-path first: x, w, grids, auxgt) ----------
            ch_t = [per.tile([128, F], f16, name=f"ch{ci}") for ci in range(5)]
            xt, yt, wt, ht, ct = ch_t
            grid_t = per.tile([128, 2 * F], f16)
            gtt = per.tile([128, 7 * MAXT], f32)
            mask_t = per.tile([128, F], f16)
            nc.scalar.dma_start(out=xt[:, :], in_=chans[0])
            nc.scalar.dma_start(out=wt[:, :], in_=chans[2])
            nc.sync.dma_start(out=grid_t[:], in_=grids[:, :])
            nc.sync.dma_start(out=gtt[:], in_=auxgt[:, :])
            sm_g, sm_a, sm_o = [], [], []
            for half in range(2):
                rows = slice(half * 128, (half + 1) * 128)
                g_t = per.tile([128, 85], f32, name=f"g_{half}")
                nc.sync.dma_start(out=g_t[:], in_=gath[rows, :])
                a_t = per.tile([128, 16], f32, name=f"a_{half}")
                nc.sync.dma_start(out=a_t[:], in_=auxc[rows, :])
                o_t = per.tile([128, NCLS], f32, name=f"o_{half}")
                nc.sync.dma_start(out=o_t[:], in_=oneh[rows, :])
                sm_g.append(g_t); sm_a.append(a_t); sm_o.append(o_t)
            nc.scalar.dma_start(out=yt[:, :], in_=chans[1])
            nc.scalar.dma_start(out=ht[:, :], in_=chans[3])
            nc.gpsimd.dma_start(out=ct[:, :], in_=chans[4])
            nc.gpsimd.dma_start(out=mask_t[:], in_=maskc[:, :])

            # ---------- hoisted per-cell quantities (fp16) ----------
            sx = per.tile([128, F], f16)
            nc.scalar.activation(sx[:], xt[:], Act.Sigmoid)
            phw = per.tile([128, F], f16)   # exp(w + ln(aw/2)) = pw/2 (host bias)
            nc.scalar.activation(phw[:], wt[:], Act.Exp)
            sy = per.tile([128, F], f16)
            nc.scalar.activation(sy[:], yt[:], Act.Sigmoid)
            phh = per.tile([128, F], f16)
            nc.scalar.activation(phh[:], ht[:], Act.Exp)
            cfs = per.tile([128, F], f16)
            nc.scalar.activation(cfs[:], ct[:], Act.Sigmoid)
            cf2 = per.tile([128, F], f16)
            nc.scalar.activation(cf2[:], cfs[:], Act.Square)

            pxc = per.tile([128, F], f16)
            nc.vector.tensor_tensor(pxc[:], sx[:], grid_t[:, 0:F], Alu.add)
            pyc = per.tile([128, F], f16)
            nc.vector.tensor_tensor(pyc[:], sy[:], grid_t[:, F:2 * F], Alu.add)
            pxr = per.tile([128, F], f16)
            nc.vector.tensor_tensor(pxr[:], pxc[:], phw[:], Alu.add)
            pxl = per.tile([128, F], f16)
            nc.vector.tensor_tensor(pxl[:], pxc[:], phw[:], Alu.subtract)
            pyr = per.tile([128, F], f16)
            nc.vector.tensor_tensor(pyr[:], pyc[:], phh[:], Alu.add)
            pyl = per.tile([128, F], f16)
            nc.vector.tensor_tensor(pyl[:], pyc[:], phh[:], Alu.subtract)

            thw = per.tile([128, F], f16)   # phw*phh (threshold/1.5)
            nc.vector.tensor_tensor(thw[:], phw[:], phh[:], Alu.mult)

            K = 4                     # gt iterations per block
            accq = per.tile([128, K * F], f16, name="accq")
            nc.gpsimd.memset(accq[:], -BIG)

            # ---------- 50-gt loop, quad-blocked ----------
            # auxgt cols: [0:50] gxr | [50:100] gxl | [100:150] -gxl
            #             [150:200] gyr | [200:250] gyl | [250:300] -0.25*ga
            # (invalid t: -1e4 | 1e4 | -1e4 | -1e4 | 1e4 | -4e4)
            # x-side shifted by gxl so the max() rides the Scalar engine:
            #   q1' = min(pxr,gxr)-gxl   m1' = relu(pxl-gxl) = max(pxl,gxl)-gxl
            #   s1 = m1'-q1' = -ox       rx = relu(-s1) = relu(ox)
            # Per-gt TS/ACT ops write F-wide slices of K*F-wide tiles; the
            # s/p/acc TTs and the rx relu then run once per block.
            for t0 in range(0, MAXT, K):
                k = min(K, MAXT - t0)
                q1 = tmp.tile([128, k * F], f16, tag="q1")
                m1 = tmp.tile([128, k * F], f16, tag="m1")
                q3 = tmp.tile([128, k * F], f16, tag="q3")
                m3 = tmp.tile([128, k * F], f16, tag="m3")
                for j in range(k):
                    t = t0 + j
                    sl = slice(j * F, (j + 1) * F)
                    nc.vector.tensor_scalar(q1[:, sl], pxr[:],
                                            gtt[:, t:t + 1],
                                            gtt[:, 50 + t:51 + t],
                                            Alu.min, Alu.subtract)
                    nc.scalar.activation(m1[:, sl], pxl[:], Act.Relu,
                                         bias=gtt[:, 100 + t:101 + t])
                    nc.vector.tensor_scalar(q3[:, sl], pyr[:],
                                            gtt[:, 150 + t:151 + t],
                                            None, Alu.min)
                    nc.vector.tensor_scalar(m3[:, sl], pyl[:],
                                            gtt[:, 200 + t:201 + t],
                                            None, Alu.max)
                s1 = tmp.tile([128, k * F], f16, tag="s1")
                nc.vector.tensor_tensor(s1[:], m1[:], q1[:], Alu.subtract)
                rx = tmp.tile([128, k * F], f16, tag="rx")  # relu(ox)
                nc.scalar.activation(rx[:], s1[:], Act.Relu, scale=-1.0)
                s3 = tmp.tile([128, k * F], f16, tag="s3")  # -oy
                nc.vector.tensor_tensor(s3[:], m3[:], q3[:], Alu.subtract)
                p = tmp.tile([128, k * F], f16, tag="p")    # -relu(ox)*oy
                nc.vector.tensor_tensor(p[:], rx[:], s3[:], Alu.mult)
                u = tmp.tile([128, k * F], f16, tag="u")    # prod - 0.375*ga
                for j in range(k):
                    t = t0 + j
                    sl = slice(j * F, (j + 1) * F)
                    nc.scalar.activation(u[:, sl], p[:, sl], Act.Identity,
                                         scale=-1.0 / 1.5,
                                         bias=gtt[:, 250 + t:251 + t])
                nc.vector.tensor_tensor(accq[:, 0:k * F], accq[:, 0:k * F],
                                        u[:], Alu.max)

            # ---------- noobj sum ----------
            acc2 = per.tile([128, 2 * F], f16)
            nc.vector.tensor_tensor(acc2[:], accq[:, 0:2 * F],
                                    accq[:, 2 * F:4 * F], Alu.max)
            accm = per.tile([128, F], f16)
            nc.vector.tensor_tensor(accm[:], acc2[:, 0:F], acc2[:, F:2 * F],
                                    Alu.max)
            cf2m = per.tile([128, F], f16)
            nc.vector.tensor_tensor(cf2m[:], cf2[:], mask_t[:], Alu.mult)
            ind = per.tile([128, F], f16)   # 1.0 where max score <= thr
            nc.vector.tensor_tensor(ind[:], accm[:], thw[:], Alu.is_le)
            contrib = per.tile([128, F], f16)
            rhs8 = per.tile([128, 8], f32)  # all partial sums, one matmul
            nc.gpsimd.memset(rhs8[:], 0.0)
            nc.vector.tensor_tensor(contrib[:], cf2m[:], ind[:], Alu.mult)
            nc.vector.tensor_reduce(rhs8[:, 0:1], contrib[:], X, Alu.add)

            ones = per.tile([128, 1], f32)
            nc.gpsimd.memset(ones[:], 1.0)

            # ---------- small stage: matched cells ----------
            for half in range(2):
                g_t, a_t, o_t = sm_g[half], sm_a[half], sm_o[half]

                # gath cols: 0 x | 1 y | 2 conf | 3 w | 4 h | 5:85 cls
                # auxc cols: 0 gi | 1 gj | 2 lnawh | 3 lnahh | 4 gxl | 5 gxr
                #            6 gyl | 7 gyr | 8 garea | 9 tx | 10 ty | 11 tw
                #            12 th | 13 valid
                sig3 = per.tile([128, 3], f32, name=f"sig3_{half}")
                nc.scalar.activation(sig3[:], g_t[:, 0:3], Act.Sigmoid)
                sphw = per.tile([128, 1], f32, name=f"sphw_{half}")
                nc.scalar.activation(sphw[:], g_t[:, 3:4], Act.Exp,
                                     bias=a_t[:, 2:3])
                sphh = per.tile([128, 1], f32, name=f"sphh_{half}")
                nc.scalar.activation(sphh[:], g_t[:, 4:5], Act.Exp,
                                     bias=a_t[:, 3:4])
                px = per.tile([128, 1], f32, name=f"px_{half}")
                nc.vector.tensor_scalar(px[:], sig3[:, 0:1], a_t[:, 0:1],
                                        None, Alu.add)
                py = per.tile([128, 1], f32, name=f"py_{half}")
                nc.vector.tensor_scalar(py[:], sig3[:, 1:2], a_t[:, 1:2],
                                        None, Alu.add)
                # overlap x
                pxr2 = per.tile([128, 1], f32, name=f"pxr2_{half}")
                nc.vector.tensor_tensor(pxr2[:], px[:], sphw[:], Alu.add)
                pxl2 = per.tile([128, 1], f32, name=f"pxl2_{half}")
                nc.vector.tensor_tensor(pxl2[:], px[:], sphw[:], Alu.subtract)
                st0 = per.tile([128, 1], f32, name=f"st0_{half}")
                nc.vector.tensor_scalar(st0[:], pxr2[:], a_t[:, 5:6],
                                        None, Alu.min)
                st1 = per.tile([128, 1], f32, name=f"st1_{half}")
                nc.vector.tensor_scalar(st1[:], pxl2[:], a_t[:, 4:5],
                                        None, Alu.max)
                sox = per.tile([128, 1], f32, name=f"sox_{half}")
                nc.vector.tensor_tensor(sox[:], st0[:], st1[:], Alu.subtract)
                soxr = per.tile([128, 1], f32, name=f"soxr_{half}")
                nc.vector.tensor_scalar(soxr[:], sox[:], 0.0, None, Alu.max)
                # overlap y
                pyr2 = per.tile([128, 1], f32, name=f"pyr2_{half}")
                nc.vector.tensor_tensor(pyr2[:], py[:], sphh[:], Alu.add)
                pyl2 = per.tile([128, 1], f32, name=f"pyl2_{half}")
                nc.vector.tensor_tensor(pyl2[:], py[:], sphh[:], Alu.subtract)
                st2 = per.tile([128, 1], f32, name=f"st2_{half}")
                nc.vector.tensor_scalar(st2[:], pyr2[:], a_t[:, 7:8],
                                        None, Alu.min)
                st3 = per.tile([128, 1], f32, name=f"st3_{half}")
                nc.vector.tensor_scalar(st3[:], pyl2[:], a_t[:, 6:7],
                                        None, Alu.max)
                soy = per.tile([128, 1], f32, name=f"soy_{half}")
                nc.vector.tensor_tensor(soy[:], st2[:], st3[:], Alu.subtract)
                soyr = per.tile([128, 1], f32, name=f"soyr_{half}")
                nc.vector.tensor_scalar(soyr[:], soy[:], 0.0, None, Alu.max)

                inter = per.tile([128, 1], f32, name=f"inter_{half}")
                nc.vector.tensor_tensor(inter[:], soxr[:], soyr[:], Alu.mult)
                pa = per.tile([128, 1], f32, name=f"pa_{half}")
                nc.vector.tensor_tensor(pa[:], sphw[:], sphh[:], Alu.mult)
                un = per.tile([128, 1], f32, name=f"un_{half}")
                nc.vector.tensor_scalar(un[:], pa[:], 4.0, a_t[:, 8:9],
                                        Alu.mult, Alu.add)
                un2 = per.tile([128, 1], f32, name=f"un2_{half}")
                nc.vector.tensor_tensor(un2[:], un[:], inter[:], Alu.subtract)
                rec = per.tile([128, 1], f32, name=f"rec_{half}")
                nc.vector.reciprocal(rec[:], un2[:])
                tiou = per.tile([128, 1], f32, name=f"tiou_{half}")
                nc.vector.tensor_tensor(tiou[:], inter[:], rec[:], Alu.mult)

                ctb = per.tile([128, 3], f32, name=f"ctb_{half}")
                # coord: 0.5*((sx-tx)^2+(sy-ty)^2+(w-tw)^2+(h-th)^2)
                scr = per.tile([128, 4], f32, name=f"scr_{half}")
                nc.vector.tensor_tensor(scr[:, 0:2], sig3[:, 0:2],
                                        a_t[:, 9:11], Alu.subtract)
                nc.vector.tensor_tensor(scr[:, 2:4], g_t[:, 3:5],
                                        a_t[:, 11:13], Alu.subtract)
                sq4 = per.tile([128, 4], f32, name=f"sq4_{half}")
                nc.scalar.activation(sq4[:], scr[:], Act.Square,
                                     scale=math.sqrt(0.5 * COORD_SCALE))
                nc.vector.tensor_reduce(ctb[:, 0:1], sq4[:], X, Alu.add)
                # obj conf: 2.5*(sconf - tiou)^2
                dcf = per.tile([128, 1], f32, name=f"dcf_{half}")
                nc.vector.tensor_tensor(dcf[:], sig3[:, 2:3], tiou[:],
                                        Alu.subtract)
                nc.scalar.activation(ctb[:, 1:2], dcf[:], Act.Square,
                                     scale=math.sqrt(0.5 * OBJ_SCALE))
                # class CE: logsumexp(cls) - <cls, onehot>
                mx = per.tile([128, 1], f32, name=f"mx_{half}")
                nc.vector.tensor_reduce(mx[:], g_t[:, 5:85], X, Alu.max)
                nmx = per.tile([128, 1], f32, name=f"nmx_{half}")
                nc.vector.tensor_scalar(nmx[:], mx[:], -1.0, None, Alu.mult)
                esc = per.tile([128, NCLS], f32, name=f"esc_{half}")
                sume = per.tile([128, 1], f32, name=f"sume_{half}")
                nc.scalar.activation(esc[:], g_t[:, 5:85], Act.Exp,
                                     bias=nmx[:])
                nc.vector.tensor_reduce(sume[:], esc[:], X, Alu.add)
                lns = per.tile([128, 1], f32, name=f"lns_{half}")
                nc.scalar.activation(lns[:], sume[:], Act.Ln)
                lse = per.tile([128, 1], f32, name=f"lse_{half}")
                nc.vector.tensor_tensor(lse[:], lns[:], mx[:], Alu.add)
                tgl = per.tile([128, NCLS], f32, name=f"tgl_{half}")
                tgv = per.tile([128, 1], f32, name=f"tgv_{half}")
                nc.vector.tensor_tensor(tgl[:], g_t[:, 5:85], o_t[:], Alu.mult)
                nc.vector.tensor_reduce(tgv[:], tgl[:], X, Alu.add)
                nc.vector.tensor_tensor(ctb[:, 2:3], lse[:], tgv[:],
                                        Alu.subtract)
                nc.vector.tensor_scalar(rhs8[:, 1 + 3 * half:4 + 3 * half],
                                        ctb[:], a_t[:, 13:14], None, Alu.mult)

            # ---------- final assembly ----------
            ps8 = ps.tile([1, 8], f32)
            nc.tensor.matmul(ps8[:], ones[:], rhs8[:], start=True, stop=True)
            out_t = per.tile([1, 16], f32)
            nc.gpsimd.memset(out_t[:], 0.0)
            nc.vector.tensor_reduce(out_t[:, 0:1], ps8[:, 0:8], X, Alu.add)
            nc.gpsimd.dma_start(out=out_d[:, :], in_=out_t[:])
    nc.finalize()
    return nc


# ---------------- host-side preparation ----------------

def _iou_np(b1, b2):
    """center-format IoU, matches reference._iou_cc; broadcastable [...,4]"""
    mx = np.minimum(b1[..., 0] - 0.5 * b1[..., 2], b2[..., 0] - 0.5 * b2[..., 2])
    Mx = np.maximum(b1[..., 0] + 0.5 * b1[..., 2], b2[..., 0] + 0.5 * b2[..., 2])
    my = np.minimum(b1[..., 1] - 0.5 * b1[..., 3], b2[..., 1] - 0.5 * b2[..., 3])
    My = np.maximum(b1[..., 1] + 0.5 * b1[..., 3], b2[..., 1] + 0.5 * b2[..., 3])
    cw = b1[..., 2] + b2[..., 2] - (Mx - mx)
    ch = b1[..., 3] + b2[..., 3] - (My - my)
    inter = np.where((cw <= 0) | (ch <= 0), 0.0, cw * ch)
    union = b1[..., 2] * b1[..., 3] + b2[..., 2] * b2[..., 3] - inter
    return inter / union


def _prep_core(out_np, tgt_np):
    """Build all device input tensors for one core (4 images)."""
    f32, f16 = np.float32, np.float16
    tgt = tgt_np.reshape(BPC, MAXT, 5).astype(f32)
    gx = tgt[:, :, 1] * NW
    gy = tgt[:, :, 2] * NH
    gw = tgt[:, :, 3] * NW
    gh = tgt[:, :, 4] * NH
    gcls = tgt[:, :, 0].astype(np.int32)
    valid = np.cumprod((tgt[:, :, 1] != 0).astype(np.int32), axis=1).astype(bool)

    # best anchor per gt by shape-only IoU (same math as reference)
    gt_shape = np.stack([np.zeros_like(gw), np.zeros_like(gw), gw, gh], -1)
    anc_box = np.stack([np.zeros(NA, f32), np.zeros(NA, f32),
                        AW.astype(f32), AH.astype(f32)], -1)
    a_ious = _iou_np(gt_shape[:, :, None, :], anc_box[None, None, :, :])
    best_n = np.argmax(a_ious, axis=-1)

    gi = gx.astype(np.int32)
    gj = gy.astype(np.int32)

    # auxgt [128, 300]: per-partition (by image) gt interval bounds
    ghw, ghh = 0.5 * gw, 0.5 * gh
    gxr = np.where(valid, gx + ghw, -1.0e4)
    gxl = np.where(valid, gx - ghw, 1.0e4)
    ngxl = np.where(valid, -(gx - ghw), -1.0e4)
    gyr = np.where(valid, gy + ghh, -1.0e4)
    gyl = np.where(valid, gy - ghh, 1.0e4)
    ncv = np.where(valid, -0.375 * gw * gh / 1.5, -BIG)
    ngyl = np.where(valid, -(gy - ghh), -1.0e4)
    aux_b = np.concatenate([gxr, gxl, ngxl, gyr, gyl, ncv, ngyl],
                           axis=1).astype(f32)
    b_of_p = np.arange(128) // PPI
    auxgt = aux_b[b_of_p]

    # chans [5, 128, F] f16: x, y, w', h', conf with w'/h' anchor-pre-biased
    out_r = out_np.reshape(BPC, NA, 85, HW).astype(f32)
    lnw = np.log(AW / 2.0).astype(f32)[None, :, None]
    lnh = np.log(AH / 2.0).astype(f32)[None, :, None]
    planes = [out_r[:, :, 0], out_r[:, :, 1],
              out_r[:, :, 2] + lnw, out_r[:, :, 3] + lnh, out_r[:, :, 4]]
    chv = np.zeros((5, 128, F), f16)
    for ci, p in enumerate(planes):
        flat = np.zeros((BPC, PPI * F), f32)
        flat[:, :NA * HW] = p.reshape(BPC, NA * HW)
        chv[ci] = flat.reshape(128, F).astype(f16)

    # grids [128, 2F]: col/row of each cell (pattern identical per image)
    k = np.arange(PPI * F)
    col = np.where(k < NA * HW, (k % HW) % NW, 2000).astype(f32)
    row = np.where(k < NA * HW, (k % HW) // NW, 2000).astype(f32)
    gim = np.concatenate([col.reshape(PPI, F), row.reshape(PPI, F)],
                         axis=1).astype(f16)
    grids = np.tile(gim, (BPC, 1))

    # scatter cells: last write wins per (b, best_n, gj, gi)
    cells = {}
    for b in range(BPC):
        for t in range(MAXT):
            if not valid[b, t]:
                continue
            key = (b, int(best_n[b, t]), int(gj[b, t]), int(gi[b, t]))
            cells[key] = t
    cell_list = list(cells.items())
    ncell = len(cell_list)
    assert ncell <= NCELL_CAP

    # maskc [128, F]: 1 on valid non-matched cells, 0 on pads & matched
    base = (0.5 * NOOBJ_SCALE * (k < NA * HW)).astype(f16).reshape(PPI, F)
    maskv = np.tile(base, (BPC, 1))
    for (b, a, j, i), _t in cell_list:
        kk = a * HW + j * NW + i
        maskv[b * PPI + kk // F, kk % F] = 0.0

    # gathered channels + per-cell aux
    gathv = np.zeros((NCELL_CAP, 85), f32)
    auxcv = np.zeros((NCELL_CAP, 16), f32)
    onehv = np.zeros((NCELL_CAP, NCLS), f32)
    auxcv[:, 8] = 1.0  # pad rows: garea=1 avoids 0-union
    for s, ((b, a, j, i), t) in enumerate(cell_list):
        hw = j * NW + i
        ch = out_r[b, a, :, hw]
        gathv[s, 0] = ch[0]
        gathv[s, 1] = ch[1]
        gathv[s, 2] = ch[4]
        gathv[s, 3] = ch[2]
        gathv[s, 4] = ch[3]
        gathv[s, 5:] = ch[5:]
        bn = a
        auxcv[s, 0] = i
        auxcv[s, 1] = j
        auxcv[s, 2] = math.log(AW[bn] / 2.0)
        auxcv[s, 3] = math.log(AH[bn] / 2.0)
        auxcv[s, 4] = gx[b, t] - 0.5 * gw[b, t]
        auxcv[s, 5] = gx[b, t] + 0.5 * gw[b, t]
        auxcv[s, 6] = gy[b, t] - 0.5 * gh[b, t]
        auxcv[s, 7] = gy[b, t] + 0.5 * gh[b, t]
        auxcv[s, 8] = gw[b, t] * gh[b, t]
        auxcv[s, 9] = gx[b, t] - float(gi[b, t])
        auxcv[s, 10] = gy[b, t] - float(gj[b, t])
        auxcv[s, 11] = math.log(gw[b, t] / AW[bn])
        auxcv[s, 12] = math.log(gh[b, t] / AH[bn])
        auxcv[s, 13] = 1.0
        onehv[s, gcls[b, t]] = 1.0

    return {
        "chans": chv, "auxgt": auxgt, "grids": grids, "maskc": maskv,
        "gath": gathv, "auxc": auxcv, "oneh": onehv,
    }


def kernel(output, target):
    from concourse.bass_utils import run_bass_kernel_spmd

    output = np.asarray(output, dtype=np.float32)
    target = np.asarray(target, dtype=np.float32)

    if "nc" not in _PROG_CACHE:
        _PROG_CACHE["nc"] = _build_program()
    nc = _PROG_CACHE["nc"]

    in_maps = []
    for core in range(NCORES):
        sl = slice(core * BPC, (core + 1) * BPC)
        in_maps.append(_prep_core(output[sl], target[sl]))

    res = run_bass_kernel_spmd(nc, in_maps, list(range(NCORES)))
    total = np.float32(0.0)
    for core in range(NCORES):
        total += np.float32(res.results[core]["out"][0, 0])
    return np.float32(total)


# revision 31
# speedup vs baseline: 1.0099x; 1.0099x over previous
"""RegionLoss (YOLOv2) Trainium2 kernel — 8-core batch-parallel SPMD.

Contract: kernel(**inputs) takes FULL inputs (output [32,425,76,76] f32,
target [32,250] f32) and returns the FULL scalar loss, matching
reference.region_loss. Internally: batch is sharded 4 images/core across 8
NeuronCores; each core computes its partial loss on device; host sums the 8
partials (the unshard step for a sum-reduced output).

Device algorithm per core (4 images):
 - Bulk stage: for every anchor cell (4*5*76*76 = 115,520 cells) compute
   sigmoid(conf)^2 * [max_t IoU(pred_cell, gt_t) <= 0.6] and sum it.
   Division-free exact test per (cell, gt), rescaled by 1/1.5 so the
   per-cell threshold is plain phw*phh:
     iou > 0.6  <=>  (relu(ox)*oy - 0.375*ga_t)/1.5 > phw*phh
   with ox = min(pxr,gxr)-max(pxl,gxl), oy likewise (exact min/max form).
   Layout: all 128 partitions, 32 per image (anchor boundaries crossed;
   the anchor log-size bias is pre-added to w/h on host), F=904 cells per
   partition, fp16. The gt loop is blocked 4-at-a-time: per gt the DVE
   runs 3 tensor_scalar min/max ops (x-side min fused with a -gxl shift
   so the x-max rides the Scalar engine as relu(pxl-gxl)), and per block
   the DVE runs 4 [128,4F] tensor_tensors (s1, s3, prod, acc-max) while
   the Scalar engine supplies relu(ox) and the threshold-subtract
   (Identity, scale=-1, per-partition bias). GpSimd stays idle during
   the loop - its SBUF port is shared with the DVE and concurrent
   streaming slows both ~5x (measured 455ns -> 2205ns per op).
 - Small stage: the <=50 matched cells per image (last-write-wins deduped
   on host) contribute coord/obj-conf/class-CE terms; the 85 channel
   values at those cells are gathered host-side (a layout/shard step -
   all math stays on device) and processed in f32 on <=256 partitions.

Everything derived from `target` (gt boxes, best anchors, scatter cells,
masks) is host-precomputed metadata passed as small input tensors, so one
input-independent NEFF serves all cores/batches.
"""

import math
import numpy as np

# ---- problem constants (hardcoded per contract) ----
NB, NH, NW = 32, 76, 76
NA, NCLS = 5, 80
MAXT = 50
ANCHORS = np.array([1.3221, 1.73145, 3.19275, 4.00944, 5.05587, 8.09892,
                    9.47112, 4.84053, 11.2364, 10.0071], dtype=np.float32)
AW = ANCHORS.reshape(NA, 2)[:, 0]
AH = ANCHORS.reshape(NA, 2)[:, 1]
COORD_SCALE, NOOBJ_SCALE, OBJ_SCALE, CLASS_SCALE = 1.0, 1.0, 5.0, 1.0
THRESH = 0.6

NCORES = 8
BPC = NB // NCORES          # 4 images per core
HW = NH * NW                # 5776
PPI = 128 // BPC            # 32 partitions per image
F = (NA * HW + PPI - 1) // PPI  # 904 cells per partition (28880 -> 28928)
NCELL_CAP = 256             # small-stage cell capacity (2 x 128)
BIG = 6.0e4                 # fp16-safe sentinel

_PROG_CACHE = {}


def _build_program():
    import concourse.bacc as bacc
    import concourse.mybir as mybir
    from concourse.tile import TileContext

    f32 = mybir.dt.float32
    f16 = mybir.dt.float16
    Alu = mybir.AluOpType
    Act = mybir.ActivationFunctionType
    X = mybir.AxisListType.X

    nc = bacc.Bacc()

    # ---- I/O ----
    chans = nc.declare_dram_parameter("chans", [5, 128, F], f16, isOutput=False)
    auxgt = nc.declare_dram_parameter("auxgt", [128, 7 * MAXT], f32, isOutput=False)
    grids = nc.declare_dram_parameter("grids", [128, 2 * F], f16, isOutput=False)
    maskc = nc.declare_dram_parameter("maskc", [128, F], f16, isOutput=False)
    gath = nc.declare_dram_parameter("gath", [NCELL_CAP, 85], f32, isOutput=False)
    auxc = nc.declare_dram_parameter("auxc", [NCELL_CAP, 16], f32, isOutput=False)
    oneh = nc.declare_dram_parameter("oneh", [NCELL_CAP, NCLS], f32, isOutput=False)
    out_d = nc.declare_dram_parameter("out", [1, 16], f32, isOutput=True)

    with TileContext(nc) as tc:
        with tc.tile_pool(name="per", bufs=1) as per, \
             tc.tile_pool(name="tmp", bufs=2) as tmp, \
             tc.tile_pool(name="ps", bufs=1, space="PSUM") as ps:

            # ---------- loads (critical-path first: x, w, grids, auxgt) ----------
            ch_t = [per.tile([128, F], f16, name=f"ch{ci}") for ci in range(5)]
            xt, yt, wt, ht, ct = ch_t
            grid_t = per.tile([128, 2 * F], f16)
            gtt = per.tile([128, 7 * MAXT], f32)
            mask_t = per.tile([128, F], f16)
            nc.scalar.dma_start(out=xt[:, :], in_=chans[0])
            nc.scalar.dma_start(out=wt[:, :], in_=chans[2])
            sm_g, sm_a, sm_o = [], [], []
            for half in range(2):
                rows = slice(half * 128, (half + 1) * 128)
                g_t = per.tile([128, 85], f32, name=f"g_{half}")
                nc.sync.dma_start(out=g_t[:], in_=gath[rows, :])
                a_t = per.tile([128, 16], f32, name=f"a_{half}")
                nc.sync.dma_start(out=a_t[:], in_=auxc[rows, :])
                o_t = per.tile([128, NCLS], f32, name=f"o_{half}")
                nc.sync.dma_start(out=o_t[:], in_=oneh[rows, :])
                sm_g.append(g_t); sm_a.append(a_t); sm_o.append(o_t)
            nc.sync.dma_start(out=grid_t[:], in_=grids[:, :])
            nc.sync.dma_start(out=gtt[:], in_=auxgt[:, :])
            nc.scalar.dma_start(out=yt[:, :], in_=chans[1])
            nc.scalar.dma_start(out=ht[:, :], in_=chans[3])
            nc.gpsimd.dma_start(out=ct[:, :], in_=chans[4])
            nc.gpsimd.dma_start(out=mask_t[:], in_=maskc[:, :])

            # ---------- hoisted per-cell quantities (fp16) ----------
            sx = per.tile([128, F], f16)
            nc.scalar.activation(sx[:], xt[:], Act.Sigmoid)
            phw = per.tile([128, F], f16)   # exp(w + ln(aw/2)) = pw/2 (host bias)
            nc.scalar.activation(phw[:], wt[:], Act.Exp)
            sy = per.tile([128, F], f16)
            nc.scalar.activation(sy[:], yt[:], Act.Sigmoid)
            phh = per.tile([128, F], f16)
            nc.scalar.activation(phh[:], ht[:], Act.Exp)
            cfs = per.tile([128, F], f16)
            nc.scalar.activation(cfs[:], ct[:], Act.Sigmoid)
            cf2 = per.tile([128, F], f16)
            nc.scalar.activation(cf2[:], cfs[:], Act.Square)

            pxc = per.tile([128, F], f16)
            nc.vector.tensor_tensor(pxc[:], sx[:], grid_t[:, 0:F], Alu.add)
            pyc = per.tile([128, F], f16)
            nc.vector.tensor_tensor(pyc[:], sy[:], grid_t[:, F:2 * F], Alu.add)
            pxr = per.tile([128, F], f16)
            nc.vector.tensor_tensor(pxr[:], pxc[:], phw[:], Alu.add)
            pxl = per.tile([128, F], f16)
            nc.vector.tensor_tensor(pxl[:], pxc[:], phw[:], Alu.subtract)
            pyr = per.tile([128, F], f16)
            nc.vector.tensor_tensor(pyr[:], pyc[:], phh[:], Alu.add)
            pyl = per.tile([128, F], f16)
            nc.vector.tensor_tensor(pyl[:], pyc[:], phh[:], Alu.subtract)

            thw = per.tile([128, F], f16)   # phw*phh (threshold/1.5)
            nc.vector.tensor_tensor(thw[:], phw[:], phh[:], Alu.mult)

            K = 4                     # gt iterations per block
            accq = per.tile([128, K * F], f16, name="accq")
            nc.gpsimd.memset(accq[:], -BIG)

            # ---------- 50-gt loop, quad-blocked ----------
            # auxgt cols: [0:50] gxr | [50:100] gxl | [100:150] -gxl
            #             [150:200] gyr | [200:250] gyl | [250:300] -0.25*ga
            # (invalid t: -1e4 | 1e4 | -1e4 | -1e4 | 1e4 | -4e4)
            # x-side shifted by gxl so the max() rides the Scalar engine:
            #   q1' = min(pxr,gxr)-gxl   m1' = relu(pxl-gxl) = max(pxl,gxl)-gxl
            #   s1 = m1'-q1' = -ox       rx = relu(-s1) = relu(ox)
            # Per-gt TS/ACT ops write F-wide slices of K*F-wide tiles; the
            # s/p/acc TTs and the rx relu then run once per block.
            for t0 in range(0, MAXT, K):
                k = min(K, MAXT - t0)
                q1 = tmp.tile([128, k * F], f16, tag="q1")
                m1 = tmp.tile([128, k * F], f16, tag="m1")
                q3 = tmp.tile([128, k * F], f16, tag="q3")
                m3 = tmp.tile([128, k * F], f16, tag="m3")
                for j in range(k):
                    t = t0 + j
                    sl = slice(j * F, (j + 1) * F)
                    nc.vector.tensor_scalar(q1[:, sl], pxr[:],
                                            gtt[:, t:t + 1],
                                            gtt[:, 50 + t:51 + t],
                                            Alu.min, Alu.subtract)
                    nc.scalar.activation(m1[:, sl], pxl[:], Act.Relu,
                                         bias=gtt[:, 100 + t:101 + t])
                    nc.vector.tensor_scalar(q3[:, sl], pyr[:],
                                            gtt[:, 150 + t:151 + t],
                                            None, Alu.min)
                    nc.vector.tensor_scalar(m3[:, sl], pyl[:],
                                            gtt[:, 200 + t:201 + t],
                                            None, Alu.max)
                s1 = tmp.tile([128, k * F], f16, tag="s1")
                nc.vector.tensor_tensor(s1[:], m1[:], q1[:], Alu.subtract)
                rx = tmp.tile([128, k * F], f16, tag="rx")  # relu(ox)
                nc.scalar.activation(rx[:], s1[:], Act.Relu, scale=-1.0)
                s3 = tmp.tile([128, k * F], f16, tag="s3")  # -oy
                nc.vector.tensor_tensor(s3[:], m3[:], q3[:], Alu.subtract)
                p = tmp.tile([128, k * F], f16, tag="p")    # -relu(ox)*oy
                nc.vector.tensor_tensor(p[:], rx[:], s3[:], Alu.mult)
                u = tmp.tile([128, k * F], f16, tag="u")    # prod - 0.375*ga
                for j in range(k):
                    t = t0 + j
                    sl = slice(j * F, (j + 1) * F)
                    nc.scalar.activation(u[:, sl], p[:, sl], Act.Identity,
                                         scale=-1.0 / 1.5,
                                         bias=gtt[:, 250 + t:251 + t])
                nc.vector.tensor_tensor(accq[:, 0:k * F], accq[:, 0:k * F],
                                        u[:], Alu.max)

            # ---------- noobj sum ----------
            acc2 = per.tile([128, 2 * F], f16)
            nc.vector.tensor_tensor(acc2[:], accq[:, 0:2 * F],
                                    accq[:, 2 * F:4 * F], Alu.max)
            accm = per.tile([128, F], f16)
            nc.vector.tensor_tensor(accm[:], acc2[:, 0:F], acc2[:, F:2 * F],
                                    Alu.max)
            cf2m = per.tile([128, F], f16)
            nc.vector.tensor_tensor(cf2m[:], cf2[:], mask_t[:], Alu.mult)
            ind = per.tile([128, F], f16)   # 1.0 where max score <= thr
            nc.vector.tensor_tensor(ind[:], accm[:], thw[:], Alu.is_le)
            contrib = per.tile([128, F], f16)
            rhs8 = per.tile([128, 8], f32)  # all partial sums, one matmul
            nc.gpsimd.memset(rhs8[:], 0.0)
            nc.vector.tensor_tensor(contrib[:], cf2m[:], ind[:], Alu.mult)
            nc.vector.tensor_reduce(rhs8[:, 0:1], contrib[:], X, Alu.add)

            ones = per.tile([128, 1], f32)
            nc.gpsimd.memset(ones[:], 1.0)

            # ---------- small stage: matched cells ----------
            for half in range(2):
                g_t, a_t, o_t = sm_g[half], sm_a[half], sm_o[half]

                # gath cols: 0 x | 1 y | 2 conf | 3 w | 4 h | 5:85 cls
                # auxc cols: 0 gi | 1 gj | 2 lnawh | 3 lnahh | 4 gxl | 5 gxr
                #            6 gyl | 7 gyr | 8 garea | 9 tx | 10 ty | 11 tw
                #            12 th | 13 valid
                sig3 = per.tile([128, 3], f32, name=f"sig3_{half}")
                nc.scalar.activation(sig3[:], g_t[:, 0:3], Act.Sigmoid)
                sphw = per.tile([128, 1], f32, name=f"sphw_{half}")
                nc.scalar.activation(sphw[:], g_t[:, 3:4], Act.Exp,
                                     bias=a_t[:, 2:3])
                sphh = per.tile([128, 1], f32, name=f"sphh_{half}")
                nc.scalar.activation(sphh[:], g_t[:, 4:5], Act.Exp,
                                     bias=a_t[:, 3:4])
                px = per.tile([128, 1], f32, name=f"px_{half}")
                nc.vector.tensor_scalar(px[:], sig3[:, 0:1], a_t[:, 0:1],
                                        None, Alu.add)
                py = per.tile([128, 1], f32, name=f"py_{half}")
                nc.vector.tensor_scalar(py[:], sig3[:, 1:2], a_t[:, 1:2],
                                        None, Alu.add)
                # overlap x
                pxr2 = per.tile([128, 1], f32, name=f"pxr2_{half}")
                nc.vector.tensor_tensor(pxr2[:], px[:], sphw[:], Alu.add)
                pxl2 = per.tile([128, 1], f32, name=f"pxl2_{half}")
                nc.vector.tensor_tensor(pxl2[:], px[:], sphw[:], Alu.subtract)
                st0 = per.tile([128, 1], f32, name=f"st0_{half}")
                nc.vector.tensor_scalar(st0[:], pxr2[:], a_t[:, 5:6],
                                        None, Alu.min)
                st1 = per.tile([128, 1], f32, name=f"st1_{half}")
                nc.vector.tensor_scalar(st1[:], pxl2[:], a_t[:, 4:5],
                                        None, Alu.max)
                sox = per.tile([128, 1], f32, name=f"sox_{half}")
                nc.vector.tensor_tensor(sox[:], st0[:], st1[:], Alu.subtract)
                soxr = per.tile([128, 1], f32, name=f"soxr_{half}")
                nc.vector.tensor_scalar(soxr[:], sox[:], 0.0, None, Alu.max)
                # overlap y
                pyr2 = per.tile([128, 1], f32, name=f"pyr2_{half}")
                nc.vector.tensor_tensor(pyr2[:], py[:], sphh[:], Alu.add)
                pyl2 = per.tile([128, 1], f32, name=f"pyl2_{half}")
                nc.vector.tensor_tensor(pyl2[:], py[:], sphh[:], Alu.subtract)
                st2 = per.tile([128, 1], f32, name=f"st2_{half}")
                nc.vector.tensor_scalar(st2[:], pyr2[:], a_t[:, 7:8],
                                        None, Alu.min)
                st3 = per.tile([128, 1], f32, name=f"st3_{half}")
                nc.vector.tensor_scalar(st3[:], pyl2[:], a_t[:, 6:7],
                                        None, Alu.max)
                soy = per.tile([128, 1], f32, name=f"soy_{half}")
                nc.vector.tensor_tensor(soy[:], st2[:], st3[:], Alu.subtract)
                soyr = per.tile([128, 1], f32, name=f"soyr_{half}")
                nc.vector.tensor_scalar(soyr[:], soy[:], 0.0, None, Alu.max)

                inter = per.tile([128, 1], f32, name=f"inter_{half}")
                nc.vector.tensor_tensor(inter[:], soxr[:], soyr[:], Alu.mult)
                pa = per.tile([128, 1], f32, name=f"pa_{half}")
                nc.vector.tensor_tensor(pa[:], sphw[:], sphh[:], Alu.mult)
                un = per.tile([128, 1], f32, name=f"un_{half}")
                nc.vector.tensor_scalar(un[:], pa[:], 4.0, a_t[:, 8:9],
                                        Alu.mult, Alu.add)
                un2 = per.tile([128, 1], f32, name=f"un2_{half}")
                nc.vector.tensor_tensor(un2[:], un[:], inter[:], Alu.subtract)
                rec = per.tile([128, 1], f32, name=f"rec_{half}")
                nc.vector.reciprocal(rec[:], un2[:])
                tiou = per.tile([128, 1], f32, name=f"tiou_{half}")
                nc.vector.tensor_tensor(tiou[:], inter[:], rec[:], Alu.mult)

                ctb = per.tile([128, 3], f32, name=f"ctb_{half}")
                # coord: 0.5*((sx-tx)^2+(sy-ty)^2+(w-tw)^2+(h-th)^2)
                scr = per.tile([128, 4], f32, name=f"scr_{half}")
                nc.vector.tensor_tensor(scr[:, 0:2], sig3[:, 0:2],
                                        a_t[:, 9:11], Alu.subtract)
                nc.vector.tensor_tensor(scr[:, 2:4], g_t[:, 3:5],
                                        a_t[:, 11:13], Alu.subtract)
                sq4 = per.tile([128, 4], f32, name=f"sq4_{half}")
                nc.scalar.activation(sq4[:], scr[:], Act.Square,
                                     scale=math.sqrt(0.5 * COORD_SCALE))
                nc.vector.tensor_reduce(ctb[:, 0:1], sq4[:], X, Alu.add)
                # obj conf: 2.5*(sconf - tiou)^2
                dcf = per.tile([128, 1], f32, name=f"dcf_{half}")
                nc.vector.tensor_tensor(dcf[:], sig3[:, 2:3], tiou[:],
                                        Alu.subtract)
                nc.scalar.activation(ctb[:, 1:2], dcf[:], Act.Square,
                                     scale=math.sqrt(0.5 * OBJ_SCALE))
                # class CE: logsumexp(cls) - <cls, onehot>
                mx = per.tile([128, 1], f32, name=f"mx_{half}")
                nc.vector.tensor_reduce(mx[:], g_t[:, 5:85], X, Alu.max)
                nmx = per.tile([128, 1], f32, name=f"nmx_{half}")
                nc.vector.tensor_scalar(nmx[:], mx[:], -1.0, None, Alu.mult)
                esc = per.tile([128, NCLS], f32, name=f"esc_{half}")
                sume = per.tile([128, 1], f32, name=f"sume_{half}")
                nc.scalar.activation(esc[:], g_t[:, 5:85], Act.Exp,
                                     bias=nmx[:])
                nc.vector.tensor_reduce(sume[:], esc[:], X, Alu.add)
                lns = per.tile([128, 1], f32, name=f"lns_{half}")
                nc.scalar.activation(lns[:], sume[:], Act.Ln)
                lse = per.tile([128, 1], f32, name=f"lse_{half}")
                nc.vector.tensor_tensor(lse[:], lns[:], mx[:], Alu.add)
                tgl = per.tile([128, NCLS], f32, name=f"tgl_{half}")
                tgv = per.tile([128, 1], f32, name=f"tgv_{half}")
                nc.vector.tensor_tensor(tgl[:], g_t[:, 5:85], o_t[:], Alu.mult)
                nc.vector.tensor_reduce(tgv[:], tgl[:], X, Alu.add)
                nc.vector.tensor_tensor(ctb[:, 2:3], lse[:], tgv[:],
                                        Alu.subtract)
                nc.vector.tensor_scalar(rhs8[:, 1 + 3 * half:4 + 3 * half],
                                        ctb[:], a_t[:, 13:14], None, Alu.mult)

            # ---------- final assembly ----------
            ps8 = ps.tile([1, 8], f32)
            nc.tensor.matmul(ps8[:], ones[:], rhs8[:], start=True, stop=True)
            out_t = per.tile([1, 16], f32)
            nc.gpsimd.memset(out_t[:], 0.0)
            nc.vector.tensor_reduce(out_t[:, 0:1], ps8[:, 0:8], X, Alu.add)
            nc.gpsimd.dma_start(out=out_d[:, :], in_=out_t[:])
    nc.finalize()
    return nc


# ---------------- host-side preparation ----------------

def _iou_np(b1, b2):
    """center-format IoU, matches reference._iou_cc; broadcastable [...,4]"""
    mx = np.minimum(b1[..., 0] - 0.5 * b1[..., 2], b2[..., 0] - 0.5 * b2[..., 2])
    Mx = np.maximum(b1[..., 0] + 0.5 * b1[..., 2], b2[..., 0] + 0.5 * b2[..., 2])
    my = np.minimum(b1[..., 1] - 0.5 * b1[..., 3], b2[..., 1] - 0.5 * b2[..., 3])
    My = np.maximum(b1[..., 1] + 0.5 * b1[..., 3], b2[..., 1] + 0.5 * b2[..., 3])
    cw = b1[..., 2] + b2[..., 2] - (Mx - mx)
    ch = b1[..., 3] + b2[..., 3] - (My - my)
    inter = np.where((cw <= 0) | (ch <= 0), 0.0, cw * ch)
    union = b1[..., 2] * b1[..., 3] + b2[..., 2] * b2[..., 3] - inter
    return inter / union


def _prep_core(out_np, tgt_np):
    """Build all device input tensors for one core (4 images)."""
    f32, f16 = np.float32, np.float16
    tgt = tgt_np.reshape(BPC, MAXT, 5).astype(f32)
    gx = tgt[:, :, 1] * NW
    gy = tgt[:, :, 2] * NH
    gw = tgt[:, :, 3] * NW
    gh = tgt[:, :, 4] * NH
    gcls = tgt[:, :, 0].astype(np.int32)
    valid = np.cumprod((tgt[:, :, 1] != 0).astype(np.int32), axis=1).astype(bool)

    # best anchor per gt by shape-only IoU (same math as reference)
    gt_shape = np.stack([np.zeros_like(gw), np.zeros_like(gw), gw, gh], -1)
    anc_box = np.stack([np.zeros(NA, f32), np.zeros(NA, f32),
                        AW.astype(f32), AH.astype(f32)], -1)
    a_ious = _iou_np(gt_shape[:, :, None, :], anc_box[None, None, :, :])
    best_n = np.argmax(a_ious, axis=-1)

    gi = gx.astype(np.int32)
    gj = gy.astype(np.int32)

    # auxgt [128, 300]: per-partition (by image) gt interval bounds
    ghw, ghh = 0.5 * gw, 0.5 * gh
    gxr = np.where(valid, gx + ghw, -1.0e4)
    gxl = np.where(valid, gx - ghw, 1.0e4)
    ngxl = np.where(valid, -(gx - ghw), -1.0e4)
    gyr = np.where(valid, gy + ghh, -1.0e4)
    gyl = np.where(valid, gy - ghh, 1.0e4)
    ncv = np.where(valid, -0.375 * gw * gh / 1.5, -BIG)
    ngyl = np.where(valid, -(gy - ghh), -1.0e4)
    aux_b = np.concatenate([gxr, gxl, ngxl, gyr, gyl, ncv, ngyl],
                           axis=1).astype(f32)
    b_of_p = np.arange(128) // PPI
    auxgt = aux_b[b_of_p]

    # chans [5, 128, F] f16: x, y, w', h', conf with w'/h' anchor-pre-biased
    out_r = out_np.reshape(BPC, NA, 85, HW).astype(f32)
    lnw = np.log(AW / 2.0).astype(f32)[None, :, None]
    lnh = np.log(AH / 2.0).astype(f32)[None, :, None]
    planes = [out_r[:, :, 0], out_r[:, :, 1],
              out_r[:, :, 2] + lnw, out_r[:, :, 3] + lnh, out_r[:, :, 4]]
    chv = np.zeros((5, 128, F), f16)
    for ci, p in enumerate(planes):
        flat = np.zeros((BPC, PPI * F), f32)
        flat[:, :NA * HW] = p.reshape(BPC, NA * HW)
        chv[ci] = flat.reshape(128, F).astype(f16)

    # grids [128, 2F]: col/row of each cell (pattern identical per image)
    k = np.arange(PPI * F)
    col = np.where(k < NA * HW, (k % HW) % NW, 2000).astype(f32)
    row = np.where(k < NA * HW, (k % HW) // NW, 2000).astype(f32)
    gim = np.concatenate([col.reshape(PPI, F), row.reshape(PPI, F)],
                         axis=1).astype(f16)
    grids = np.tile(gim, (BPC, 1))

    # scatter cells: last write wins per (b, best_n, gj, gi)
    cells = {}
    for b in range(BPC):
        for t in range(MAXT):
            if not valid[b, t]:
                continue
            key = (b, int(best_n[b, t]), int(gj[b, t]), int(gi[b, t]))
            cells[key] = t
    cell_list = list(cells.items())
    ncell = len(cell_list)
    assert ncell <= NCELL_CAP

    # maskc [128, F]: 1 on valid non-matched cells, 0 on pads & matched
    base = (0.5 * NOOBJ_SCALE * (k < NA * HW)).astype(f16).reshape(PPI, F)
    maskv = np.tile(base, (BPC, 1))
    for (b, a, j, i), _t in cell_list:
        kk = a * HW + j * NW + i
        maskv[b * PPI + kk // F, kk % F] = 0.0

    # gathered channels + per-cell aux
    gathv = np.zeros((NCELL_CAP, 85), f32)
    auxcv = np.zeros((NCELL_CAP, 16), f32)
    onehv = np.zeros((NCELL_CAP, NCLS), f32)
    auxcv[:, 8] = 1.0  # pad rows: garea=1 avoids 0-union
    for s, ((b, a, j, i), t) in enumerate(cell_list):
        hw = j * NW + i
        ch = out_r[b, a, :, hw]
        gathv[s, 0] = ch[0]
        gathv[s, 1] = ch[1]
        gathv[s, 2] = ch[4]
        gathv[s, 3] = ch[2]
        gathv[s, 4] = ch[3]
        gathv[s, 5:] = ch[5:]
        bn = a
        auxcv[s, 0] = i
        auxcv[s, 1] = j
        auxcv[s, 2] = math.log(AW[bn] / 2.0)
        auxcv[s, 3] = math.log(AH[bn] / 2.0)
        auxcv[s, 4] = gx[b, t] - 0.5 * gw[b, t]
        auxcv[s, 5] = gx[b, t] + 0.5 * gw[b, t]
        auxcv[s, 6] = gy[b, t] - 0.5 * gh[b, t]
        auxcv[s, 7] = gy[b, t] + 0.5 * gh[b, t]
        auxcv[s, 8] = gw[b, t] * gh[b, t]
        auxcv[s, 9] = gx[b, t] - float(gi[b, t])
        auxcv[s, 10] = gy[b, t] - float(gj[b, t])
        auxcv[s, 11] = math.log(gw[b, t] / AW[bn])
        auxcv[s, 12] = math.log(gh[b, t] / AH[bn])
        auxcv[s, 13] = 1.0
        onehv[s, gcls[b, t]] = 1.0

    return {
        "chans": chv, "auxgt": auxgt, "grids": grids, "maskc": maskv,
        "gath": gathv, "auxc": auxcv, "oneh": onehv,
    }


def kernel(output, target):
    from concourse.bass_utils import run_bass_kernel_spmd

    output = np.asarray(output, dtype=np.float32)
    target = np.asarray(target, dtype=np.float32)

    if "nc" not in _PROG_CACHE:
        _PROG_CACHE["nc"] = _build_program()
    nc = _PROG_CACHE["nc"]

    in_maps = []
    for core in range(NCORES):
        sl = slice(core * BPC, (core + 1) * BPC)
        in_maps.append(_prep_core(output[sl], target[sl]))

    res = run_bass_kernel_spmd(nc, in_maps, list(range(NCORES)))
    total = np.float32(0.0)
    for core in range(NCORES):
        total += np.float32(res.results[core]["out"][0, 0])
    return np.float32(total)
